# revision 8
# baseline (speedup 1.0000x reference)
import sys

sys.path.insert(0, '/opt/trn_rl_repo')

import numpy as np
import concourse.bass as bass
import concourse.mybir as mybir
import concourse.tile as tile
from concourse import bacc, bass_utils

F32 = mybir.dt.float32
F32R = mybir.dt.float32r
AF = mybir.ActivationFunctionType

D_MODEL = 1024
N_HEADS = 16
D_HEAD = 64
SEQ = 2048
BATCH = 2
N_CORES = 8
HPC = 4            # heads per core
CSL = HPC * D_HEAD  # 256: qkv feature slice per core
OSL = D_MODEL // 4  # 256: output column slice per core
NT_S = SEQ // 128   # 16
NT_D = D_MODEL // 128  # 8
NQ = SEQ // 512     # 4 q-tiles
GROUPS = [[0, 1, 2, 3], [4, 5, 6, 7]]

_cache = {}


def _build():
    nc = bacc.Bacc("TRN2", target_bir_lowering=False, debug=False,
                   num_devices=N_CORES)
    x_in = nc.dram_tensor("x", [SEQ, D_MODEL], F32, kind="ExternalInput").ap()
    wq_in = nc.dram_tensor("wq", [D_MODEL, CSL], F32, kind="ExternalInput").ap()
    wk_in = nc.dram_tensor("wk", [D_MODEL, CSL], F32, kind="ExternalInput").ap()
    wv_in = nc.dram_tensor("wv", [D_MODEL, CSL], F32, kind="ExternalInput").ap()
    wo_in = nc.dram_tensor("wo", [D_MODEL, OSL], F32, kind="ExternalInput").ap()
    bq_in = nc.dram_tensor("bq", [1, CSL], F32, kind="ExternalInput").ap()
    bk_in = nc.dram_tensor("bk", [1, CSL], F32, kind="ExternalInput").ap()
    bv_in = nc.dram_tensor("bv", [1, CSL], F32, kind="ExternalInput").ap()
    bo_in = nc.dram_tensor("bo", [1, OSL], F32, kind="ExternalInput").ap()
    id_in = nc.dram_tensor("ident", [128, 128], F32, kind="ExternalInput").ap()
    nm_in = nc.dram_tensor("negmask", [128, 128], F32, kind="ExternalInput").ap()
    or_in = nc.dram_tensor("ones_row", [1, 512], F32, kind="ExternalInput").ap()
    oc_in = nc.dram_tensor("ones_col", [1, 128], F32, kind="ExternalInput").ap()
    out = nc.dram_tensor("out", [SEQ, OSL], F32, kind="ExternalOutput").ap()

    with tile.TileContext(nc) as tc:
        _body(nc, tc, x_in, wq_in, wk_in, wv_in, wo_in, bq_in, bk_in, bv_in,
              bo_in, id_in, nm_in, or_in, oc_in, out)
    nc.compile()
    return nc


def _body(nc, tc, x_in, wq_in, wk_in, wv_in, wo_in, bq_in, bk_in, bv_in,
          bo_in, id_in, nm_in, or_in, oc_in, out):
    from contextlib import ExitStack
    ctx = ExitStack()
    with ctx:
        const = ctx.enter_context(tc.tile_pool(name="const", bufs=1))
        wopool = ctx.enter_context(tc.tile_pool(name="wopool", bufs=1))
        drpool = ctx.enter_context(tc.tile_pool(name="drpool", bufs=1, space="DRAM"))
        mid = ctx.enter_context(ExitStack())
        qkpool = mid.enter_context(tc.tile_pool(name="qkpool", bufs=1))
        vpool = mid.enter_context(tc.tile_pool(name="vpool", bufs=1))
        htpool = mid.enter_context(tc.tile_pool(name="htpool", bufs=1))
        s1 = ExitStack()
        wpool = s1.enter_context(tc.tile_pool(name="wpool", bufs=1))

        # ---- constants ----
        ident = const.tile([128, 128], F32R)
        nc.sync.dma_start(ident[:], id_in.bitcast(F32R))
        negm = const.tile([128, 128], F32)
        nc.sync.dma_start(negm[:], nm_in[:])
        ones_row = const.tile([1, 512], F32R)
        nc.sync.dma_start(ones_row[:], or_in.bitcast(F32R))
        ones_col = const.tile([1, 128], F32R)
        nc.sync.dma_start(ones_col[:], oc_in.bitcast(F32R))
        bq_t = const.tile([1, CSL], F32R)
        nc.sync.dma_start(bq_t[:], bq_in.bitcast(F32R))
        bk_t = const.tile([1, CSL], F32R)
        nc.sync.dma_start(bk_t[:], bk_in.bitcast(F32R))
        bv_t = const.tile([1, CSL], F32R)
        nc.sync.dma_start(bv_t[:], bv_in.bitcast(F32R))
        bo_t = const.tile([1, OSL], F32R)
        nc.sync.dma_start(bo_t[:], bo_in.bitcast(F32R))

        # ---- weights: [D, C] -> sbuf [128, NT_D*C] (col block t = d-tile t) ----
        def load_w(ap_in, cols):
            t = wpool.tile([128, NT_D * cols], F32R, name=f"w_{ap_in.tensor.name}")
            src = ap_in.bitcast(F32R).rearrange("(t p) c -> p t c", p=128)
            dst = t.rearrange("p (t c) -> p t c", c=cols)
            nc.sync.dma_start(dst, src)
            return t

        wq_t = load_w(wq_in, CSL)
        wk_t = load_w(wk_in, CSL)
        wv_t = load_w(wv_in, CSL)

        wo_t = wopool.tile([128, NT_D * OSL], F32R, name="w_wo")
        nc.sync.dma_start(
            wo_t.rearrange("p (t c) -> p t c", c=OSL),
            wo_in.bitcast(F32R).rearrange("(t p) c -> p t c", p=128))

        # ---- stage A: x -> xT (PE transpose) ----
        xtpool = s1.enter_context(tc.tile_pool(name="xtpool", bufs=1))
        xT = []
        for dj in range(NT_D):
            t = xtpool.tile([128, SEQ], F32R, name=f"xT{dj}", tag=f"xT{dj}")
            xT.append(t)
        with tc.tile_pool(name="xstage", bufs=3) as xpool, \
             tc.tile_pool(name="ps_tr", bufs=4, space="PSUM") as ps_tr:
            for si in range(NT_S):
                xs = xpool.tile([128, D_MODEL], F32R, name=f"xs{si}", tag="xs")
                nc.sync.dma_start(
                    xs[:], x_in.bitcast(F32R)[128 * si:128 * (si + 1), :])
                for dj in range(NT_D):
                    pt = ps_tr.tile([128, 128], F32R, name=f"pt{si}_{dj}", tag="pt")
                    nc.tensor.transpose(
                        pt[:], xs[:, 128 * dj:128 * (dj + 1)], ident[:])
                    nc.vector.tensor_copy(
                        xT[dj][:, 128 * si:128 * (si + 1)], pt[:])

        # ---- stage B: QKV projections ----
        qt, kt = [], []
        for ci in range(2):
            qt.append(qkpool.tile([128, SEQ], F32R, name=f"qt{ci}", tag=f"qt{ci}"))
            kt.append(qkpool.tile([128, SEQ], F32R, name=f"kt{ci}", tag=f"kt{ci}"))
        vt = [vpool.tile([128, HPC * 65], F32R, name=f"vt{si}", tag=f"vt{si}")
              for si in range(NT_S)]

        with tc.tile_pool(name="ps_qkv", bufs=3, space="PSUM") as ps_qkv:
            for (w_t, b_t, dst) in ((wq_t, bq_t, qt), (wk_t, bk_t, kt)):
                for ci in range(2):
                    for sj in range(NQ):
                        pp = ps_qkv.tile([128, 512], F32, name="pp", tag="qk")
                        for dj in range(NT_D):
                            nc.tensor.matmul(
                                pp[:],
                                w_t[:, dj * CSL + 128 * ci:dj * CSL + 128 * (ci + 1)],
                                xT[dj][:, 512 * sj:512 * (sj + 1)],
                                start=(dj == 0), stop=False)
                        nc.tensor.matmul(
                            pp[:], b_t[0:1, 128 * ci:128 * (ci + 1)], ones_row[:],
                            start=False, stop=True)
                        nc.vector.tensor_copy(
                            dst[ci][:, 512 * sj:512 * (sj + 1)], pp[:])
            for si in range(NT_S):
                pv = ps_qkv.tile([128, CSL], F32, name="pv", tag="v")
                for dj in range(NT_D):
                    nc.tensor.matmul(
                        pv[:],
                        xT[dj][:, 128 * si:128 * (si + 1)],
                        wv_t[:, dj * CSL:(dj + 1) * CSL],
                        start=(dj == 0), stop=False)
                nc.tensor.matmul(pv[:], ones_col[:], bv_t[:],
                                 start=False, stop=True)
                nc.vector.memset(vt[si].bitcast(F32)[:], 1.0)
                nc.vector.tensor_copy(
                    vt[si].rearrange("p (h e) -> p h e", e=65)[:, :, 0:64],
                    pv.rearrange("p (h e) -> p h e", e=64)[:, :, :])
        s1.close()

        # ---- stage C: attention ----
        hT = [htpool.tile([128, SEQ], F32R, name=f"hT{ci}", tag=f"hT{ci}")
              for ci in range(2)]
        with tc.tile_pool(name="ps_s", bufs=1, space="PSUM") as ps_s, \
             tc.tile_pool(name="ps_att", bufs=2, space="PSUM") as ps_att, \
             tc.tile_pool(name="ps_bc", bufs=1, space="PSUM") as ps_bc, \
             tc.tile_pool(name="exp_pool", bufs=3) as exp_pool, \
             tc.tile_pool(name="misc_c", bufs=2) as misc_c:
            for ci in range(2):
                for qi in range(NQ):
                    nk = 4 * qi + 4
                    pa = [ps_att.tile([65, 512], F32, name=f"pa{hh}", tag=f"att{hh}")
                          for hh in range(2)]
                    for ki in range(nk):
                        r = ki - 4 * qi
                        c0 = 0 if r < 0 else 128 * r
                        for hh in range(2):
                            p0 = 64 * hh
                            h_local = 2 * ci + hh
                            pscr = ps_s.tile([128, 512], F32, name=f"ps{hh}",
                                             tag=f"s{hh}")
                            nc.tensor.matmul(
                                pscr[:],
                                kt[ci][p0:p0 + 64, 128 * ki:128 * (ki + 1)],
                                qt[ci][p0:p0 + 64, 512 * qi:512 * (qi + 1)],
                                start=True, stop=True)
                            if r >= 0:
                                nc.vector.tensor_add(
                                    pscr[:, c0:c0 + 128], pscr[:, c0:c0 + 128],
                                    negm[:])
                            et = exp_pool.tile([128, 512], F32R, name=f"et{hh}",
                                               tag=f"e{hh}")
                            nc.scalar.activation(
                                et[:, c0:512], pscr[:, c0:512], AF.Exp)
                            nc.tensor.matmul(
                                pa[hh][:, c0:512],
                                vt[ki][:, 65 * h_local:65 * h_local + 65],
                                et[:, c0:512],
                                start=(ki == 0), stop=(ki == nk - 1),
                                skip_group_check=True)
                    rt0 = misc_c.tile([1, 512], F32, name="rt0", tag="rt0")
                    rt1 = misc_c.tile([1, 512], F32, name="rt1", tag="rt1")
                    nc.vector.reciprocal(rt0[:], pa[0][64:65, :])
                    nc.vector.reciprocal(rt1[:], pa[1][64:65, :])
                    pb = ps_bc.tile([128, 512], F32, name="pb", tag="bc")
                    ocf = ones_col.bitcast(F32)
                    nc.tensor.matmul(pb[0:64, :], ocf[0:1, 0:64], rt0[:],
                                     start=True, stop=True)
                    nc.tensor.matmul(pb[64:128, :], ocf[0:1, 0:64], rt1[:],
                                     start=True, stop=True, tile_position=(0, 64))
                    bc = misc_c.tile([128, 512], F32, name="bc", tag="bc")
                    nc.scalar.copy(bc[:], pb[:])
                    for hh in range(2):
                        nc.vector.tensor_mul(
                            hT[ci][64 * hh:64 * (hh + 1),
                                   512 * qi:512 * (qi + 1)],
                            pa[hh][0:64, :], bc[64 * hh:64 * (hh + 1), :])

        # ---- stage D: AllGather + output projection ----
        hbounce = drpool.tile([2 * 128, SEQ], F32R, name="hbounce")
        for ci in range(2):
            nc.sync.dma_start(hbounce[128 * ci:128 * (ci + 1), :], hT[ci][:])
        gout = drpool.tile([4 * 2 * 128, SEQ], F32R, name="gout")
        nc.gpsimd.collective_compute(
            "AllGather", mybir.AluOpType.bypass, replica_groups=GROUPS,
            ins=[hbounce.opt()], outs=[gout.opt()])
        mid.close()

        with tc.tile_pool(name="gpool", bufs=1) as gpool, \
             tc.tile_pool(name="ostage", bufs=4) as ostage, \
             tc.tile_pool(name="ps_out", bufs=3, space="PSUM") as ps_out:
            ga = []
            for ct in range(NT_D):
                g = gpool.tile([128, SEQ], F32R, name=f"ga{ct}", tag=f"ga{ct}")
                nc.sync.dma_start(g[:], gout[128 * ct:128 * (ct + 1), :])
                ga.append(g)
            for si in range(NT_S):
                po = ps_out.tile([128, OSL], F32, name="po", tag="po")
                for ct in range(NT_D):
                    nc.tensor.matmul(
                        po[:],
                        ga[ct][:, 128 * si:128 * (si + 1)],
                        wo_t[:, OSL * ct:OSL * (ct + 1)],
                        start=(ct == 0), stop=False)
                nc.tensor.matmul(po[:], ones_col[:], bo_t[:],
                                 start=False, stop=True)
                so = ostage.tile([128, OSL], F32, name="so", tag="so")
                nc.scalar.copy(so[:], po[:])
                nc.sync.dma_start(out[128 * si:128 * (si + 1), :], so[:])


def _consts():
    ident = np.eye(128, dtype=np.float32)
    kk = np.arange(128)[:, None]
    qq = np.arange(128)[None, :]
    negmask = np.where(kk <= qq, 0.0, -1e30).astype(np.float32)
    ones_row = np.ones((1, 512), dtype=np.float32)
    ones_col = np.ones((1, 128), dtype=np.float32)
    return ident, negmask, ones_row, ones_col


def kernel(x, Wq, bq, Wk, bk, Wv, bv, Wo, bo):
    x = np.asarray(x, dtype=np.float32)
    Wq = np.asarray(Wq, dtype=np.float32)
    bq = np.asarray(bq, dtype=np.float32)
    Wk = np.asarray(Wk, dtype=np.float32)
    bk = np.asarray(bk, dtype=np.float32)
    Wv = np.asarray(Wv, dtype=np.float32)
    bv = np.asarray(bv, dtype=np.float32)
    Wo = np.asarray(Wo, dtype=np.float32)
    bo = np.asarray(bo, dtype=np.float32)

    if "nc" not in _cache:
        _cache["nc"] = _build()
    nc = _cache["nc"]

    ident, negmask, ones_row, ones_col = _consts()
    scale = 1.0 / np.sqrt(np.float32(D_HEAD))
    in_maps = []
    for core in range(N_CORES):
        b, g = divmod(core, HPC)
        csl = slice(CSL * g, CSL * (g + 1))
        osl = slice(OSL * g, OSL * (g + 1))
        in_maps.append({
            "x": np.ascontiguousarray(x[b]),
            "wq": np.ascontiguousarray(Wq[:, csl] * scale),
            "wk": np.ascontiguousarray(Wk[:, csl]),
            "wv": np.ascontiguousarray(Wv[:, csl]),
            "wo": np.ascontiguousarray(Wo[:, osl]),
            "bq": np.ascontiguousarray(bq[None, csl] * scale),
            "bk": np.ascontiguousarray(bk[None, csl]),
            "bv": np.ascontiguousarray(bv[None, csl]),
            "bo": np.ascontiguousarray(bo[None, osl]),
            "ident": ident, "negmask": negmask, "ones_row": ones_row,
            "ones_col": ones_col,
        })

    res = bass_utils.run_bass_kernel_spmd(
        nc, in_maps, core_ids=list(range(N_CORES)))

    full = np.empty((BATCH, SEQ, D_MODEL), dtype=np.float32)
    for core in range(N_CORES):
        b, g = divmod(core, HPC)
        full[b, :, OSL * g:OSL * (g + 1)] = res.results[core]["out"]
    return full
